# revision 15
# baseline (speedup 1.0000x reference)
"""Causal multi-head self-attention on 8 Trainium2 NeuronCores.

Sharding: tensor-parallel over heads. 16 heads / 8 cores = 2 heads per core.
Each core computes the QKV projection for its 2 heads (full sequence, both
batches), causal flash-style attention for its 2 heads, and a partial output
projection against its slice of W_o columns. The host sums the 8 partial
outputs (the "all-reduce" of the tensor-parallel scheme, done during unshard).

Matmul inputs are fp16 (PE streams 1 row/cycle vs 4 for fp32; fp16 keeps
11 mantissa bits vs bf16's 8), accumulation is always fp32 in PSUM, softmax
runs in fp32. End-to-end error vs the fp32 reference is ~4e-4 relative.

Device layout (contraction dim always on partitions):
  - x passed pre-transposed and pre-cast: xT [D, B*S] fp16.
  - Projection computes Q^T/K^T/V^T [128=2*dk, S] per batch directly.
  - Scores computed transposed, S^T[k, q] = K^T.T @ Q^T (fp32 PSUM), both
    heads into one [128, 2, 512] PSUM tile via separate PE row groups (the
    two matmuls run concurrently in different PE row strips).
  - One ACT exp per score tile (PSUM -> SBUF fp16), causal diagonal blocks
    column-sliced, the remaining 128-band masked with a triangular multiply.
  - V^T transposed on-PE to V[tok, dv] with a ones column appended, so the
    AV matmul also accumulates the softmax row-sums (row 64 of the output).
  - Normalization: stage O^T/row-sum to SBUF (frees PSUM), GPSIMD
    partition-broadcast of the row-sum (base-0 output only: HW ignores the
    out AP base), ~51ULP reciprocal, DVE multiply into mhaT fp16.
  - Output projection: out[tok,:] = mhaT_tile.T @ WoT, fp32 result to DRAM.

The emission order software-pipelines the batches: batch1's projection is
interleaved into batch0's attention (attention is exp/ACT-gated, leaving PE
slack), and batch0's output projection into batch1's attention.
"""

import numpy as np

import concourse.bacc as bacc
import concourse.mybir as mybir
import concourse.tile as tile

FP32 = mybir.dt.float32
FP16 = mybir.dt.float16

B = 2
S = 2048
D = 1024
NUM_HEADS = 16
DK = 64
NCORES = 8
HPC = NUM_HEADS // NCORES  # heads per core = 2
HD = HPC * DK  # 128, head dims per core

QCW = 512  # q chunk width
KTW = 128  # k tile width (partition dim)

NP_IN = np.float16


def build_nc(d=D, s=S, b=B):
    """Build the per-core Bass program. All 8 cores run this same program."""
    assert d % 128 == 0 and s % QCW == 0 and QCW % KTW == 0
    ndc = d // 128  # d_model chunks
    nqc = s // QCW  # q chunks per batch
    nkt = s // KTW  # k tiles per batch
    kpq = QCW // KTW  # k tiles per q chunk (4)
    ntt = s // 128  # token tiles per batch

    nc = bacc.Bacc("TRN2", target_bir_lowering=False)

    xT_d = nc.dram_tensor("xT", [d, b * s], FP16, kind="ExternalInput")
    wt_d = nc.dram_tensor("wqkvT", [d, 3 * HD], FP16, kind="ExternalInput")
    wo_d = nc.dram_tensor("woT", [HD, d], FP16, kind="ExternalInput")
    tri_d = nc.dram_tensor("tri", [128, 128], FP16, kind="ExternalInput")
    id_d = nc.dram_tensor("ident", [128, 128], FP16, kind="ExternalInput")
    out_d = nc.dram_tensor("out", [b * s, d], FP32, kind="ExternalOutput")

    with tile.TileContext(nc) as tc:
        with (
            tc.tile_pool(name="consts", bufs=1) as consts,
            tc.tile_pool(name="xts", bufs=b * ndc) as xts_pool,
            tc.tile_pool(name="qkv", bufs=2) as qkv_pool,
            tc.tile_pool(name="vsb", bufs=2) as v_pool,
            tc.tile_pool(name="pt", bufs=3) as pt_pool,
            tc.tile_pool(name="mha", bufs=2) as mha_pool,
            tc.tile_pool(name="osb", bufs=3) as out_pool,
            tc.tile_pool(name="small", bufs=2) as small_pool,
            tc.tile_pool(name="ps_mm", bufs=2, space="PSUM") as ps_mm,
            tc.tile_pool(name="ps_s", bufs=2, space="PSUM") as ps_s,
            tc.tile_pool(name="ps_o", bufs=1, space="PSUM") as ps_o,
        ):
            # ---- input loads: weights for the first projection, then x of
            # batch 0, then the small attention constants, then x of batch 1
            wt_sb = consts.tile([128, ndc, 3 * HD], FP16)
            for k in range(ndc):
                nc.sync.dma_start(wt_sb[:, k, :], wt_d[128 * k : 128 * (k + 1), :])
            xts_all = []
            for bi in range(b):
                xts_all.append(
                    [
                        xts_pool.tile([128, s], FP16, name=f"xt{bi}_{k}", tag="xt")
                        for k in range(ndc)
                    ]
                )
            for k in range(ndc):
                nc.sync.dma_start(
                    xts_all[0][k], xT_d[128 * k : 128 * (k + 1), 0:s]
                )
            tri_sb = consts.tile([128, 128], FP16)
            nc.sync.dma_start(tri_sb, tri_d[:, :])
            id_sb = consts.tile([128, 128], FP16)
            nc.sync.dma_start(id_sb, id_d[:, :])
            wo_sb = consts.tile([128, d], FP16)
            nc.sync.dma_start(wo_sb, wo_d[:, :])
            for bi in range(1, b):
                for k in range(ndc):
                    nc.sync.dma_start(
                        xts_all[bi][k],
                        xT_d[128 * k : 128 * (k + 1), bi * s : (bi + 1) * s],
                    )

            qkvTs = [qkv_pool.tile([128, 3, s], FP16, name=f"qkvT{bi}", tag="qkvT")
                     for bi in range(b)]
            v_sbs = [v_pool.tile([128, nkt, 2 * (DK + 1)], FP16, name=f"v{bi}",
                                 tag="vsb") for bi in range(b)]
            mhaTs = [mha_pool.tile([128, s], FP16, name=f"mhaT{bi}", tag="mhaT")
                     for bi in range(b)]

            def emit_proj_group(bi, m, n):
                qkvT, xts = qkvTs[bi], xts_all[bi]
                pp = ps_mm.tile([128, QCW], FP32, name="pp", tag="mm")
                for k in range(ndc):
                    nc.tensor.matmul(
                        pp,
                        wt_sb[:, k, 128 * m : 128 * (m + 1)],
                        xts[k][:, QCW * n : QCW * (n + 1)],
                        start=(k == 0),
                        stop=(k == ndc - 1),
                    )
                nc.vector.tensor_copy(qkvT[:, m, QCW * n : QCW * (n + 1)], pp)

            def emit_vsb_init(bi):
                nc.gpsimd.memset(v_sbs[bi], 1.0)

            def emit_trans(bi, t):
                qkvT, v_sb = qkvTs[bi], v_sbs[bi]
                tp = ps_mm.tile([128, 128], FP16, name="tp", tag="mm")
                nc.tensor.transpose(tp, qkvT[:, 2, 128 * t : 128 * (t + 1)], id_sb)
                nc.vector.tensor_copy(v_sb[:, t, 0:DK], tp[:, 0:DK])
                nc.vector.tensor_copy(
                    v_sb[:, t, DK + 1 : 2 * DK + 1], tp[:, DK : 2 * DK]
                )

            def emit_attn_chunk(bi, qc, filler):
                """One q-chunk of attention for batch bi. Calls filler() after
                each k-tile block to interleave independent PE work."""
                qkvT, v_sb, mhaT = qkvTs[bi], v_sbs[bi], mhaTs[bi]
                q0 = QCW * qc
                oA = ps_o.tile([DK + 1, QCW], FP32, name="oA", tag="oA")
                oB = ps_o.tile([DK + 1, QCW], FP32, name="oB", tag="oB")
                kts = kpq * (qc + 1)
                for kt in range(kts):
                    c0 = KTW * (kt - kpq * qc) if kt >= kpq * qc else 0
                    sp = ps_s.tile([128, 2, QCW], FP32, name="sp", tag="s")
                    # scores S^T[k, q]; head A rows 0:64, head B rows 64:128
                    nc.tensor.matmul(
                        sp[:, 0, c0:QCW],
                        qkvT[0:DK, 1, KTW * kt : KTW * (kt + 1)],
                        qkvT[0:DK, 0, q0 + c0 : q0 + QCW],
                    )
                    nc.tensor.matmul(
                        sp[:, 1, c0:QCW],
                        qkvT[DK : 2 * DK, 1, KTW * kt : KTW * (kt + 1)],
                        qkvT[DK : 2 * DK, 0, q0 + c0 : q0 + QCW],
                    )
                    pt = pt_pool.tile([128, 2, QCW], FP16, name="pt", tag="pt")
                    nc.scalar.activation(
                        pt[:, :, c0:QCW],
                        sp[:, :, c0:QCW],
                        mybir.ActivationFunctionType.Exp,
                    )
                    if kt >= kpq * qc:
                        # triangular mask on the diagonal 128-band
                        nc.vector.tensor_mul(
                            pt[:, 0, c0 : c0 + KTW], pt[:, 0, c0 : c0 + KTW], tri_sb
                        )
                        nc.vector.tensor_mul(
                            pt[:, 1, c0 : c0 + KTW], pt[:, 1, c0 : c0 + KTW], tri_sb
                        )
                    nc.tensor.matmul(
                        oA[:, c0:QCW],
                        v_sb[:, kt, 0 : DK + 1],
                        pt[:, 0, c0:QCW],
                        start=(kt == 0),
                        stop=(kt == kts - 1),
                    )
                    nc.tensor.matmul(
                        oB[:, c0:QCW],
                        v_sb[:, kt, DK + 1 : 2 * DK + 2],
                        pt[:, 1, c0:QCW],
                        start=(kt == 0),
                        stop=(kt == kts - 1),
                    )
                    filler()
                # normalize: stage O^T + row-sum to base-0 SBUF (frees PSUM),
                # broadcast row-sum (base-0 out only), reciprocal, multiply
                for h, oh in ((0, oA), (1, oB)):
                    ost = small_pool.tile([DK, QCW], FP32, name="ost", tag=f"ost{h}")
                    nc.vector.tensor_copy(ost, oh[0:DK, :])
                    t = small_pool.tile([1, QCW], FP32, name="t", tag=f"t{h}")
                    nc.vector.tensor_copy(t, oh[DK : DK + 1, :])
                    bc = small_pool.tile([DK, QCW], FP32, name="bc", tag=f"bc{h}")
                    nc.gpsimd.partition_broadcast(bc, t, channels=DK)
                    nc.vector.reciprocal_approx_fast(out=bc, in_=bc)
                    nc.vector.tensor_mul(
                        mhaT[DK * h : DK * (h + 1), q0 : q0 + QCW], ost, bc
                    )

            def emit_fp_tile(bi, t):
                mhaT = mhaTs[bi]
                fps = []
                for half in range(d // QCW):
                    fp = ps_mm.tile([128, QCW], FP32, name="fp", tag="mm")
                    nc.tensor.matmul(
                        fp,
                        mhaT[:, 128 * t : 128 * (t + 1)],
                        wo_sb[:, QCW * half : QCW * (half + 1)],
                    )
                    fps.append(fp)
                ob = out_pool.tile([128, d], FP32, name="ob", tag="ob")
                for half in range(d // QCW):
                    nc.vector.tensor_copy(
                        ob[:, QCW * half : QCW * (half + 1)], fps[half]
                    )
                r0 = bi * s + 128 * t
                nc.sync.dma_start(out_d[r0 : r0 + 128, :], ob)

            def make_filler(items):
                """Returns a filler() that emits one queued item per call,
                paced so the queue drains evenly over the attention blocks."""
                state = {"i": 0}

                def filler():
                    if state["i"] < len(items):
                        items[state["i"]]()
                        state["i"] += 1

                def flush():
                    while state["i"] < len(items):
                        items[state["i"]]()
                        state["i"] += 1

                return filler, flush

            # ---- phase A: batch0 projection + V transpose ----
            for m in range(3):
                for n in range(nqc):
                    emit_proj_group(0, m, n)
            emit_vsb_init(0)
            for t in range(nkt):
                emit_trans(0, t)

            # ---- attention phases: batch bi's attention runs with batch
            # bi+1's projection and batch bi-1's out-projection as PE filler
            for bi in range(b):
                items = []
                if bi + 1 < b:
                    items += [
                        (lambda m=m, n=n, bj=bi + 1: emit_proj_group(bj, m, n))
                        for m in range(3)
                        for n in range(nqc)
                    ]
                    items.append(lambda bj=bi + 1: emit_vsb_init(bj))
                    items += [
                        (lambda t=t, bj=bi + 1: emit_trans(bj, t))
                        for t in range(nkt)
                    ]
                if bi - 1 >= 0:
                    items += [
                        (lambda t=t, bj=bi - 1: emit_fp_tile(bj, t))
                        for t in range(ntt)
                    ]
                filler, flush = make_filler(items)
                for qc in range(nqc):
                    emit_attn_chunk(bi, qc, filler)
                flush()

            # ---- final out-proj for the last batch ----
            for t in range(ntt):
                emit_fp_tile(b - 1, t)

    nc.compile()
    return nc


def make_core_inputs(x, W_qkv, W_o, d=D, s=S, b=B):
    """Host-side shard prep. Returns list of per-core input dicts."""
    nh = W_qkv.shape[0] // (3 * DK)
    xT = np.ascontiguousarray(
        x.astype(np.float32).transpose(2, 0, 1).reshape(d, b * s).astype(NP_IN)
    )
    tri = np.triu(np.ones((128, 128), dtype=NP_IN))  # tri[k,q]=1 iff q>=k
    ident = np.eye(128, dtype=NP_IN)
    scale = np.float32(1.0 / np.sqrt(DK))
    in_maps = []
    for c in range(NCORES):
        h0 = HPC * c
        r = slice(h0 * DK, (h0 + HPC) * DK)
        wq = W_qkv[0 * nh * DK :][r] * scale
        wk = W_qkv[1 * nh * DK :][r]
        wv = W_qkv[2 * nh * DK :][r]
        ws = np.concatenate([wq, wk, wv], axis=0)  # [3*HD, d]
        wT = np.ascontiguousarray(ws.T.astype(NP_IN))  # [d, 3*HD]
        woT = np.ascontiguousarray(W_o[:, r].T.astype(NP_IN))  # [HD, d]
        in_maps.append(
            {"xT": xT, "wqkvT": wT, "woT": woT, "tri": tri, "ident": ident}
        )
    return in_maps


_NC_CACHE = {}


def kernel(x, W_qkv, W_o):
    from concourse.bass_utils import run_bass_kernel_spmd

    b, s, d = x.shape
    if "nc" not in _NC_CACHE:
        _NC_CACHE["nc"] = build_nc(d=d, s=s, b=b)
    nc = _NC_CACHE["nc"]
    in_maps = make_core_inputs(x, W_qkv, W_o, d=d, s=s, b=b)
    res = run_bass_kernel_spmd(nc, in_maps, core_ids=list(range(NCORES)))
    out = res.results[0]["out"].astype(np.float64)
    for c in range(1, NCORES):
        out += res.results[c]["out"]
    return out.astype(np.float32).reshape(b, s, d)


# revision 18
# speedup vs baseline: 1.0705x; 1.0705x over previous
"""Causal multi-head self-attention on 8 Trainium2 NeuronCores.

Sharding: tensor-parallel over heads. 16 heads / 8 cores = 2 heads per core.
Each core computes the QKV projection for its 2 heads (full sequence, both
batches), causal flash-style attention for its 2 heads, and a partial output
projection against its slice of W_o columns. The host sums the 8 partial
outputs (the "all-reduce" of the tensor-parallel scheme, done during unshard).

Matmul inputs are fp16 (PE streams 1 row/cycle vs 4 for fp32; fp16 keeps
11 mantissa bits vs bf16's 8), accumulation is always fp32 in PSUM, softmax
runs in fp32. End-to-end error vs the fp32 reference is ~4e-4 relative.

Device layout (contraction dim always on partitions):
  - x passed pre-transposed and pre-cast: xT [D, B*S] fp16.
  - Projection computes Q^T/K^T/V^T [128=2*dk, S] per batch directly.
  - Scores computed transposed, S^T[k, q] = K^T.T @ Q^T (fp32 PSUM), both
    heads into one [128, 2, 512] PSUM tile via separate PE row groups (the
    two matmuls run concurrently in different PE row strips).
  - One ACT exp per score tile (PSUM -> SBUF fp16), causal diagonal blocks
    column-sliced, the remaining 128-band masked with a triangular multiply.
  - V^T transposed on-PE to V[tok, dv] with a ones column appended, so the
    AV matmul also accumulates the softmax row-sums (row 64 of the output).
  - Normalization: stage O^T/row-sum to SBUF (frees PSUM), GPSIMD
    partition-broadcast of the row-sum (base-0 output only: HW ignores the
    out AP base), ~51ULP reciprocal, DVE multiply into mhaT fp16.
  - Output projection: out[tok,:] = mhaT_tile.T @ WoT, fp32 result to DRAM.

The emission order software-pipelines the batches: batch1's projection is
interleaved into batch0's attention (attention is exp/ACT-gated, leaving PE
slack), and batch0's output projection into batch1's attention.
"""

import numpy as np

import concourse.bacc as bacc
import concourse.mybir as mybir
import concourse.tile as tile

FP32 = mybir.dt.float32
FP16 = mybir.dt.float16

B = 2
S = 2048
D = 1024
NUM_HEADS = 16
DK = 64
NCORES = 8
HPC = NUM_HEADS // NCORES  # heads per core = 2
HD = HPC * DK  # 128, head dims per core

QCW = 512  # q chunk width
KTW = 128  # k tile width (partition dim)

NP_IN = np.float16


def build_nc(d=D, s=S, b=B):
    """Build the per-core Bass program. All 8 cores run this same program."""
    assert d % 128 == 0 and s % QCW == 0 and QCW % KTW == 0
    ndc = d // 128  # d_model chunks
    nqc = s // QCW  # q chunks per batch
    nkt = s // KTW  # k tiles per batch
    kpq = QCW // KTW  # k tiles per q chunk (4)
    ntt = s // 128  # token tiles per batch

    nc = bacc.Bacc("TRN2", target_bir_lowering=False)

    xT_d = nc.dram_tensor("xT", [d, b * s], FP16, kind="ExternalInput")
    wt_d = nc.dram_tensor("wqkvT", [d, 3 * HD], FP16, kind="ExternalInput")
    wo_d = nc.dram_tensor("woT", [HD, d], FP16, kind="ExternalInput")
    tri_d = nc.dram_tensor("tri", [128, 128], FP16, kind="ExternalInput")
    id_d = nc.dram_tensor("ident", [128, 128], FP16, kind="ExternalInput")
    out_d = nc.dram_tensor("out", [b * s, d], FP32, kind="ExternalOutput")

    with tile.TileContext(nc) as tc:
        with (
            tc.tile_pool(name="consts", bufs=1) as consts,
            tc.tile_pool(name="xts", bufs=b * ndc) as xts_pool,
            tc.tile_pool(name="qkv", bufs=2) as qkv_pool,
            tc.tile_pool(name="vsb", bufs=2) as v_pool,
            tc.tile_pool(name="pt", bufs=3) as pt_pool,
            tc.tile_pool(name="mha", bufs=2) as mha_pool,
            tc.tile_pool(name="osb", bufs=3) as out_pool,
            tc.tile_pool(name="small", bufs=2) as small_pool,
            tc.tile_pool(name="ps_mm", bufs=2, space="PSUM") as ps_mm,
            tc.tile_pool(name="ps_s", bufs=2, space="PSUM") as ps_s,
            tc.tile_pool(name="ps_o", bufs=1, space="PSUM") as ps_o,
        ):
            # ---- input loads: weights for the first projection, then x of
            # batch 0, then the small attention constants, then x of batch 1
            wt_sb = consts.tile([128, ndc, 3 * HD], FP16)
            for k in range(ndc):
                nc.sync.dma_start(wt_sb[:, k, :], wt_d[128 * k : 128 * (k + 1), :])
            xts_all = []
            for bi in range(b):
                xts_all.append(
                    [
                        xts_pool.tile([128, s], FP16, name=f"xt{bi}_{k}", tag="xt")
                        for k in range(ndc)
                    ]
                )
            for k in range(ndc):
                nc.sync.dma_start(
                    xts_all[0][k], xT_d[128 * k : 128 * (k + 1), 0:s]
                )
            tri_sb = consts.tile([128, 128], FP16)
            nc.sync.dma_start(tri_sb, tri_d[:, :])
            id_sb = consts.tile([128, 128], FP16)
            nc.sync.dma_start(id_sb, id_d[:, :])
            wo_sb = consts.tile([128, d], FP16)
            nc.sync.dma_start(wo_sb, wo_d[:, :])
            for bi in range(1, b):
                for k in range(ndc):
                    nc.sync.dma_start(
                        xts_all[bi][k],
                        xT_d[128 * k : 128 * (k + 1), bi * s : (bi + 1) * s],
                    )

            qkvTs = [qkv_pool.tile([128, 3, s], FP16, name=f"qkvT{bi}", tag="qkvT")
                     for bi in range(b)]
            v_sbs = [v_pool.tile([128, nkt, 2 * (DK + 1)], FP16, name=f"v{bi}",
                                 tag="vsb") for bi in range(b)]
            mhaTs = [mha_pool.tile([128, s], FP16, name=f"mhaT{bi}", tag="mhaT")
                     for bi in range(b)]

            def emit_proj_group(bi, m, n):
                qkvT, xts = qkvTs[bi], xts_all[bi]
                pp = ps_mm.tile([128, QCW], FP32, name="pp", tag="mm")
                for k in range(ndc):
                    nc.tensor.matmul(
                        pp,
                        wt_sb[:, k, 128 * m : 128 * (m + 1)],
                        xts[k][:, QCW * n : QCW * (n + 1)],
                        start=(k == 0),
                        stop=(k == ndc - 1),
                    )
                # ACT is otherwise idle during projection phases; it also
                # casts fp32 PSUM -> fp16 SBUF on the way out.
                nc.scalar.copy(qkvT[:, m, QCW * n : QCW * (n + 1)], pp)

            def emit_vsb_init(bi):
                nc.gpsimd.memset(v_sbs[bi], 1.0)

            def emit_trans(bi, t):
                qkvT, v_sb = qkvTs[bi], v_sbs[bi]
                tp = ps_mm.tile([128, 128], FP16, name="tp", tag="mm")
                nc.tensor.transpose(tp, qkvT[:, 2, 128 * t : 128 * (t + 1)], id_sb)
                nc.vector.tensor_copy(v_sb[:, t, 0:DK], tp[:, 0:DK])
                nc.vector.tensor_copy(
                    v_sb[:, t, DK + 1 : 2 * DK + 1], tp[:, DK : 2 * DK]
                )

            def emit_attn_chunk(bi, qc, filler):
                """One q-chunk of attention for batch bi. Calls filler() after
                each k-tile block to interleave independent PE work."""
                qkvT, v_sb, mhaT = qkvTs[bi], v_sbs[bi], mhaTs[bi]
                q0 = QCW * qc
                oA = ps_o.tile([DK + 1, QCW], FP32, name="oA", tag="oA")
                oB = ps_o.tile([DK + 1, QCW], FP32, name="oB", tag="oB")
                kts = kpq * (qc + 1)
                for kt in range(kts):
                    c0 = KTW * (kt - kpq * qc) if kt >= kpq * qc else 0
                    sp = ps_s.tile([128, 2, QCW], FP32, name="sp", tag="s")
                    # scores S^T[k, q]; head A rows 0:64, head B rows 64:128
                    nc.tensor.matmul(
                        sp[:, 0, c0:QCW],
                        qkvT[0:DK, 1, KTW * kt : KTW * (kt + 1)],
                        qkvT[0:DK, 0, q0 + c0 : q0 + QCW],
                    )
                    nc.tensor.matmul(
                        sp[:, 1, c0:QCW],
                        qkvT[DK : 2 * DK, 1, KTW * kt : KTW * (kt + 1)],
                        qkvT[DK : 2 * DK, 0, q0 + c0 : q0 + QCW],
                    )
                    pt = pt_pool.tile([128, 2, QCW], FP16, name="pt", tag="pt")
                    nc.scalar.activation(
                        pt[:, :, c0:QCW],
                        sp[:, :, c0:QCW],
                        mybir.ActivationFunctionType.Exp,
                    )
                    if kt >= kpq * qc:
                        # triangular mask on the diagonal 128-band
                        nc.vector.tensor_mul(
                            pt[:, 0, c0 : c0 + KTW], pt[:, 0, c0 : c0 + KTW], tri_sb
                        )
                        nc.vector.tensor_mul(
                            pt[:, 1, c0 : c0 + KTW], pt[:, 1, c0 : c0 + KTW], tri_sb
                        )
                    nc.tensor.matmul(
                        oA[:, c0:QCW],
                        v_sb[:, kt, 0 : DK + 1],
                        pt[:, 0, c0:QCW],
                        start=(kt == 0),
                        stop=(kt == kts - 1),
                    )
                    nc.tensor.matmul(
                        oB[:, c0:QCW],
                        v_sb[:, kt, DK + 1 : 2 * DK + 2],
                        pt[:, 1, c0:QCW],
                        start=(kt == 0),
                        stop=(kt == kts - 1),
                    )
                    filler()
                # normalize: stage O^T + row-sum to base-0 SBUF (frees PSUM),
                # broadcast row-sum (base-0 out only), reciprocal, multiply
                for h, oh in ((0, oA), (1, oB)):
                    ost = small_pool.tile([DK, QCW], FP32, name="ost", tag=f"ost{h}")
                    nc.vector.tensor_copy(ost, oh[0:DK, :])
                    t = small_pool.tile([1, QCW], FP32, name="t", tag=f"t{h}")
                    nc.vector.tensor_copy(t, oh[DK : DK + 1, :])
                    bc = small_pool.tile([DK, QCW], FP32, name="bc", tag=f"bc{h}")
                    nc.gpsimd.partition_broadcast(bc, t, channels=DK)
                    nc.vector.reciprocal_approx_fast(out=bc, in_=bc)
                    nc.vector.tensor_mul(
                        mhaT[DK * h : DK * (h + 1), q0 : q0 + QCW], ost, bc
                    )

            def emit_fp_tile(bi, t):
                mhaT = mhaTs[bi]
                fps = []
                for half in range(d // QCW):
                    fp = ps_mm.tile([128, QCW], FP32, name="fp", tag="mm")
                    nc.tensor.matmul(
                        fp,
                        mhaT[:, 128 * t : 128 * (t + 1)],
                        wo_sb[:, QCW * half : QCW * (half + 1)],
                    )
                    fps.append(fp)
                ob = out_pool.tile([128, d], FP32, name="ob", tag="ob")
                for half in range(d // QCW):
                    nc.vector.tensor_copy(
                        ob[:, QCW * half : QCW * (half + 1)], fps[half]
                    )
                r0 = bi * s + 128 * t
                nc.sync.dma_start(out_d[r0 : r0 + 128, :], ob)

            # ---- per batch: projection + V transpose (PE-dense), then
            # attention with the batch's own out-projection tiles emitted
            # right after each q-chunk normalizes (fills the PE slack of the
            # exp/ACT-bound attention phase; aux PSUM slots are free then)
            tpq = ntt // nqc  # out-proj token tiles ready per q-chunk
            for bi in range(b):
                for m in range(3):
                    for n in range(nqc):
                        emit_proj_group(bi, m, n)
                emit_vsb_init(bi)
                for t in range(nkt):
                    emit_trans(bi, t)
                for qc in range(nqc):
                    emit_attn_chunk(bi, qc, lambda: None)
                    for t in range(tpq * qc, tpq * (qc + 1)):
                        emit_fp_tile(bi, t)

    nc.compile()
    return nc


def make_core_inputs(x, W_qkv, W_o, d=D, s=S, b=B):
    """Host-side shard prep. Returns list of per-core input dicts."""
    nh = W_qkv.shape[0] // (3 * DK)
    xT = np.ascontiguousarray(
        x.astype(np.float32).transpose(2, 0, 1).reshape(d, b * s).astype(NP_IN)
    )
    tri = np.triu(np.ones((128, 128), dtype=NP_IN))  # tri[k,q]=1 iff q>=k
    ident = np.eye(128, dtype=NP_IN)
    scale = np.float32(1.0 / np.sqrt(DK))
    in_maps = []
    for c in range(NCORES):
        h0 = HPC * c
        r = slice(h0 * DK, (h0 + HPC) * DK)
        wq = W_qkv[0 * nh * DK :][r] * scale
        wk = W_qkv[1 * nh * DK :][r]
        wv = W_qkv[2 * nh * DK :][r]
        ws = np.concatenate([wq, wk, wv], axis=0)  # [3*HD, d]
        wT = np.ascontiguousarray(ws.T.astype(NP_IN))  # [d, 3*HD]
        woT = np.ascontiguousarray(W_o[:, r].T.astype(NP_IN))  # [HD, d]
        in_maps.append(
            {"xT": xT, "wqkvT": wT, "woT": woT, "tri": tri, "ident": ident}
        )
    return in_maps


_NC_CACHE = {}


def kernel(x, W_qkv, W_o):
    from concourse.bass_utils import run_bass_kernel_spmd

    b, s, d = x.shape
    if "nc" not in _NC_CACHE:
        _NC_CACHE["nc"] = build_nc(d=d, s=s, b=b)
    nc = _NC_CACHE["nc"]
    in_maps = make_core_inputs(x, W_qkv, W_o, d=d, s=s, b=b)
    res = run_bass_kernel_spmd(nc, in_maps, core_ids=list(range(NCORES)))
    out = res.results[0]["out"].astype(np.float64)
    for c in range(1, NCORES):
        out += res.results[c]["out"]
    return out.astype(np.float32).reshape(b, s, d)
